# revision 5
# baseline (speedup 1.0000x reference)
"""Trainium2 Bass kernel for nn_Head (NTM-style addressing head).

Data-parallel over batch: 8 cores x 128 samples. Each core computes its
[128, 16384] slice of the output with zero collectives.

Self-contained: only imports numpy + the installed concourse stack.
"""

import sys

if "/opt/trn_rl_repo" not in sys.path:
    sys.path.insert(0, "/opt/trn_rl_repo")

from contextlib import ExitStack

import numpy as np

import concourse.bass as bass
import concourse.tile as tile
from concourse import bacc, mybir
from concourse.bass_utils import run_bass_kernel_spmd
from concourse.masks import make_identity

B, H, N, M = 1024, 512, 16384, 64
NCORES = 8
BL = B // NCORES  # 128 samples per core == partition count
HALF = N // 2  # 8192
NCH = 8  # elementwise chunks over N
CH = N // NCH  # 2048
F32 = mybir.dt.float32
AF = mybir.ActivationFunctionType
OP = mybir.AluOpType
AX = mybir.AxisListType
GMAX = 1.0 - 2.0**-23  # clamp for g so (1-g) stays representable


def _body(ctx: ExitStack, tc: tile.TileContext, out, h, wprev, m, wcat, bcat):
    nc = tc.nc

    const = ctx.enter_context(tc.tile_pool(name="const", bufs=1))
    small = ctx.enter_context(tc.tile_pool(name="small", bufs=1))
    mpool = ctx.enter_context(tc.tile_pool(name="mpool", bufs=2))
    big = ctx.enter_context(tc.tile_pool(name="big", bufs=1))
    lpool = ctx.enter_context(tc.tile_pool(name="lpool", bufs=1))
    psum = ctx.enter_context(tc.tile_pool(name="psum", bufs=2, space="PSUM"))

    # ---- constants ----
    ident = const.tile([128, 128], F32, name="ident")
    make_identity(nc, ident)
    bias_sb = const.tile([128, 70], F32, name="bias_sb")
    nc.gpsimd.dma_start(out=bias_sb, in_=bcat.to_broadcast((128, 70)))
    wcat_sb = const.tile([128, 4, 70], F32, name="wcat_sb")
    nc.sync.dma_start(out=wcat_sb, in_=wcat.rearrange("(j p) c -> p j c", p=128))

    # ---- h -> hT (4 PE transposes packed into one PSUM tile) ----
    h_sb = small.tile([128, H], F32, name="h_sb")
    nc.sync.dma_start(out=h_sb, in_=h)
    ps_h = psum.tile([128, CH], F32, name="ps_h", tag="ps")
    for j in range(4):
        nc.tensor.transpose(
            ps_h[:, 128 * j : 128 * (j + 1)], h_sb[:, 128 * j : 128 * (j + 1)], ident
        )
    hT = const.tile([128, 512], F32, name="hT")
    nc.scalar.copy(hT, ps_h[:, 0:512])

    # ---- fused head projections: proj = h @ Wcat + bcat  [128, 70] ----
    ps_p = psum.tile([128, CH], F32, name="ps_p", tag="ps")
    for j in range(4):
        nc.tensor.matmul(
            ps_p[:, 0:70],
            lhsT=hT[:, 128 * j : 128 * (j + 1)],
            rhs=wcat_sb[:, j, :],
            start=(j == 0),
            stop=(j == 3),
        )
    proj = small.tile([128, 70], F32, name="proj")
    nc.vector.tensor_tensor(out=proj, in0=ps_p[:, 0:70], in1=bias_sb, op=OP.add)

    # ---- head activations ----
    k_sb = small.tile([128, 64], F32, name="k_sb")
    nc.vector.tensor_scalar(
        out=k_sb, in0=proj[:, 0:64], scalar1=0.0, scalar2=1.0, op0=OP.max, op1=OP.min
    )
    beta = small.tile([128, 1], F32, name="beta")
    nc.vector.tensor_scalar(
        out=beta, in0=proj[:, 64:65], scalar1=0.0, scalar2=None, op0=OP.max
    )
    g_sb = small.tile([128, 1], F32, name="g_sb")
    nc.vector.tensor_scalar(
        out=g_sb, in0=proj[:, 65:66], scalar1=0.0, scalar2=1.0, op0=OP.max, op1=OP.min
    )
    gamma = small.tile([128, 1], F32, name="gamma")
    nc.vector.tensor_scalar(
        out=gamma, in0=proj[:, 69:70], scalar1=0.0, scalar2=1.0, op0=OP.max, op1=OP.add
    )
    # shift softmax over 3 logits (small range -> no max subtraction)
    es = small.tile([128, 3], F32, name="es")
    ssum = small.tile([128, 1], F32, name="ssum")
    nc.scalar.activation(out=es, in_=proj[:, 66:69], func=AF.Exp, accum_out=ssum)
    lss = small.tile([128, 1], F32, name="lss")
    nc.scalar.activation(out=lss, in_=ssum, func=AF.Ln)
    iss = small.tile([128, 1], F32, name="iss")
    nc.scalar.activation(out=iss, in_=lss, func=AF.Exp, scale=-1.0)
    s3 = small.tile([128, 3], F32, name="s3")
    nc.vector.tensor_scalar(out=s3, in0=es, scalar1=iss, scalar2=None, op0=OP.mult)

    # ---- bscale = beta / ||k|| ----
    scratch64 = small.tile([128, 64], F32, name="scratch64")
    ksq = small.tile([128, 1], F32, name="ksq")
    nc.scalar.activation(out=scratch64, in_=k_sb, func=AF.Square, accum_out=ksq)
    kln = small.tile([128, 1], F32, name="kln")
    nc.scalar.activation(out=kln, in_=ksq, func=AF.Ln)
    invk = small.tile([128, 1], F32, name="invk")
    nc.scalar.activation(out=invk, in_=kln, func=AF.Exp, scale=-0.5)
    bscale = small.tile([128, 1], F32, name="bscale")
    nc.vector.tensor_tensor(out=bscale, in0=beta, in1=invk, op=OP.mult)

    # ---- kT, duplicated on both partition halves: transpose [k | k] ----
    k2 = small.tile([128, 128], F32, name="k2")
    nc.vector.tensor_copy(k2[:, 0:64], k_sb)
    nc.vector.tensor_copy(k2[:, 64:128], k_sb)
    ps_k = psum.tile([128, CH], F32, name="ps_k", tag="ps")
    nc.tensor.transpose(ps_k[:, 0:128], k2, ident)
    kT = const.tile([128, 128], F32, name="kT")
    nc.scalar.copy(kT, ps_k[:, 0:128])

    # ---- m_t: normalize rows and transpose into mT [128, 8192]
    #      rows 0:64  = m^T for n in [0, 8192)
    #      rows 64:128= m^T for n in [8192, 16384) ----
    mT = big.tile([128, HALF], F32, name="mT")
    for e in range(4):
        pe = mpool.tile([128, 16, 2, 64], F32, name="mpair", tag="mpair")
        lo = m[e * 2048 : (e + 1) * 2048, :].rearrange("(c p) m -> p c m", p=128)
        hi = m[HALF + e * 2048 : HALF + (e + 1) * 2048, :].rearrange(
            "(c p) m -> p c m", p=128
        )
        nc.sync.dma_start(out=pe[:, :, 0, :], in_=lo)
        nc.sync.dma_start(out=pe[:, :, 1, :], in_=hi)
        msq = mpool.tile([128, 16, 2, 64], F32, name="msq", tag="msq", bufs=1)
        nc.gpsimd.tensor_tensor(out=msq, in0=pe, in1=pe, op=OP.mult)
        nrm2 = mpool.tile([128, 16, 2], F32, name="nrm2", tag="nrm2")
        nc.vector.tensor_reduce(out=nrm2, in_=msq, axis=AX.X, op=OP.add)
        lnn = mpool.tile([128, 16, 2], F32, name="lnn", tag="lnn")
        nc.scalar.activation(out=lnn, in_=nrm2, func=AF.Ln)
        invn = mpool.tile([128, 16, 2], F32, name="invn", tag="invn")
        nc.scalar.activation(out=invn, in_=lnn, func=AF.Exp, scale=-0.5)
        nc.vector.tensor_tensor(
            out=pe, in0=pe, in1=invn.broadcast_to((128, 16, 2, 64)), op=OP.mult
        )
        ps_m = psum.tile([128, CH], F32, name="ps_m", tag="ps")
        for c in range(16):
            nc.tensor.transpose(ps_m[:, c * 128 : (c + 1) * 128], pe[:, c, :, :], ident)
        if e % 2 == 0:
            nc.vector.tensor_copy(mT[:, e * 2048 : (e + 1) * 2048], ps_m)
        else:
            nc.scalar.copy(mT[:, e * 2048 : (e + 1) * 2048], ps_m)

    # ---- w_tm1 in one big DMA ----
    wp_sb = big.tile([128, N], F32, name="wp_sb")
    nc.sync.dma_start(out=wp_sb, in_=wprev)

    # ---- content scores + exp (u = exp(beta * cos)), row sums via accum ----
    u_sb = big.tile([128, N], F32, name="u_sb")
    racc = small.tile([128, NCH], F32, name="racc")
    for j in range(NCH):
        ps_n = psum.tile([128, CH], F32, name="ps_n", tag="ps")
        if j < 4:
            kTv, rows, base = kT[0:64, :], slice(0, 64), j * CH
        else:
            kTv, rows, base = kT[64:128, :], slice(64, 128), (j - 4) * CH
        for b in range(4):
            nc.tensor.matmul(
                ps_n[:, b * 512 : (b + 1) * 512],
                lhsT=kTv,
                rhs=mT[rows, base + b * 512 : base + (b + 1) * 512],
                start=True,
                stop=True,
            )
        nc.scalar.activation(
            out=u_sb[:, j * CH : (j + 1) * CH],
            in_=ps_n,
            func=AF.Exp,
            scale=bscale,
            accum_out=racc[:, j : j + 1],
        )

    # ---- R = sum(u); gs = g / ((1-g_clamped) * R) ----
    R = small.tile([128, 1], F32, name="R")
    nc.vector.tensor_reduce(out=R, in_=racc, axis=AX.X, op=OP.add)
    gcl = small.tile([128, 1], F32, name="gcl")
    nc.vector.tensor_scalar(out=gcl, in0=g_sb, scalar1=GMAX, scalar2=None, op0=OP.min)
    om = small.tile([128, 1], F32, name="om")
    nc.vector.tensor_scalar(
        out=om, in0=gcl, scalar1=-1.0, scalar2=1.0, op0=OP.mult, op1=OP.add
    )
    omr = small.tile([128, 1], F32, name="omr")
    nc.vector.tensor_tensor(out=omr, in0=om, in1=R, op=OP.mult)
    lomr = small.tile([128, 1], F32, name="lomr")
    nc.scalar.activation(out=lomr, in_=omr, func=AF.Ln)
    iomr = small.tile([128, 1], F32, name="iomr")
    nc.scalar.activation(out=iomr, in_=lomr, func=AF.Exp, scale=-1.0)
    gs = small.tile([128, 1], F32, name="gs")
    nc.vector.tensor_tensor(out=gs, in0=g_sb, in1=iomr, op=OP.mult)

    # ---- v_hat = gs*u + w_tm1   (in place over w_tm1) ----
    for j in range(NCH):
        sl = slice(j * CH, (j + 1) * CH)
        nc.vector.scalar_tensor_tensor(
            out=wp_sb[:, sl],
            in0=u_sb[:, sl],
            scalar=gs,
            in1=wp_sb[:, sl],
            op0=OP.mult,
            op1=OP.add,
        )

    # ---- per-sample diag(s_j) stationaries for the shift conv ----
    diag = const.tile([128, 3, 128], F32, name="diag")
    for t in range(3):
        nc.vector.tensor_scalar(
            out=diag[:, t, :], in0=ident, scalar1=s3[:, t : t + 1], scalar2=None,
            op0=OP.mult,
        )

    # ---- conv (PE, accumulating taps) -> ln -> exp(gamma*...) ----
    w_sb = u_sb  # reuse: u dead after v_hat
    sacc = small.tile([128, NCH], F32, name="sacc")
    eps_b = small.tile([128, 1], F32, name="eps_b")
    nc.vector.memset(eps_b, 1e-6)
    for j in range(NCH):
        ps_c = psum.tile([128, CH], F32, name="ps_c", tag="ps")
        # tap order [1, 0, 2]: tap 1 (no shift) covers every column -> start
        for ti, t in enumerate([1, 0, 2]):
            for b in range(4):
                o0, o1 = b * 512, (b + 1) * 512
                r0 = j * CH + b * 512 + t - 1
                r1 = r0 + 512
                if r0 < 0:
                    o0, r0 = o0 + 1, 0
                if r1 > N:
                    o1, r1 = o1 - 1, N
                nc.tensor.matmul(
                    ps_c[:, o0:o1],
                    lhsT=diag[:, t, :],
                    rhs=wp_sb[:, r0:r1],
                    start=(ti == 0),
                    stop=(ti == 2),
                    skip_group_check=True,
                )
        lsc = lpool.tile([128, CH], F32, name="lsc", tag="lsc")
        nc.scalar.activation(out=lsc, in_=ps_c, func=AF.Ln, scale=om, bias=eps_b)
        nc.scalar.activation(
            out=w_sb[:, j * CH : (j + 1) * CH],
            in_=lsc,
            func=AF.Exp,
            scale=gamma,
            accum_out=sacc[:, j : j + 1],
        )

    # ---- final normalize + store ----
    Ssum = small.tile([128, 1], F32, name="Ssum")
    nc.vector.tensor_reduce(out=Ssum, in_=sacc, axis=AX.X, op=OP.add)
    lS = small.tile([128, 1], F32, name="lS")
    nc.scalar.activation(out=lS, in_=Ssum, func=AF.Ln)
    sinv = small.tile([128, 1], F32, name="sinv")
    nc.scalar.activation(out=sinv, in_=lS, func=AF.Exp, scale=-1.0)
    for j in range(NCH):
        sl = slice(j * CH, (j + 1) * CH)
        nc.gpsimd.tensor_scalar(
            out=w_sb[:, sl], in0=w_sb[:, sl], scalar1=sinv, scalar2=None, op0=OP.mult
        )
        nc.sync.dma_start(out=out[:, sl], in_=w_sb[:, sl])


def build_program():
    nc = bacc.Bacc(
        "TRN2", target_bir_lowering=False, debug=False, num_devices=NCORES
    )
    h = nc.dram_tensor("h", [BL, H], F32, kind="ExternalInput").ap()
    wprev = nc.dram_tensor("wprev", [BL, N], F32, kind="ExternalInput").ap()
    m = nc.dram_tensor("m", [N, M], F32, kind="ExternalInput").ap()
    wcat = nc.dram_tensor("wcat", [H, 70], F32, kind="ExternalInput").ap()
    bcat = nc.dram_tensor("bcat", [1, 70], F32, kind="ExternalInput").ap()
    out = nc.dram_tensor("out", [BL, N], F32, kind="ExternalOutput").ap()
    with tile.TileContext(nc) as tc, ExitStack() as ctx:
        _body(ctx, tc, out, h, wprev, m, wcat, bcat)
    nc.compile()
    return nc


_CACHED_NC = None


def _pack_host_inputs(
    h_t, w_tm1, m_t, Wk, bk, Wb, bb, Wg, bg, Ws, bs, Wm, bm
):
    wcat = np.concatenate([Wk, Wb, Wg, Ws, Wm], axis=0).astype(np.float32)  # [70, H]
    bcat = np.concatenate([bk, bb, bg, bs, bm], axis=0).astype(np.float32)  # [70]
    wcat_t = np.ascontiguousarray(wcat.T)  # [H, 70]
    bcat2 = np.ascontiguousarray(bcat.reshape(1, 70))
    m_c = np.ascontiguousarray(np.asarray(m_t, dtype=np.float32))
    in_maps = []
    for c in range(NCORES):
        sl = slice(c * BL, (c + 1) * BL)
        in_maps.append(
            {
                "h": np.ascontiguousarray(np.asarray(h_t[sl], dtype=np.float32)),
                "wprev": np.ascontiguousarray(np.asarray(w_tm1[sl], dtype=np.float32)),
                "m": m_c,
                "wcat": wcat_t,
                "bcat": bcat2,
            }
        )
    return in_maps


def kernel(**inputs) -> np.ndarray:
    global _CACHED_NC
    if _CACHED_NC is None:
        _CACHED_NC = build_program()
    in_maps = _pack_host_inputs(**inputs)
    res = run_bass_kernel_spmd(_CACHED_NC, in_maps, core_ids=list(range(NCORES)))
    return np.concatenate([res.results[c]["out"] for c in range(NCORES)], axis=0)


if __name__ == "__main__":
    rng = np.random.default_rng(0)
    ins = {
        "h_t": rng.standard_normal((B, H), dtype=np.float32),
        "w_tm1": rng.random((B, N), dtype=np.float32),
        "m_t": rng.random((N, M), dtype=np.float32),
        "Wk": rng.standard_normal((M, H), dtype=np.float32) * 0.04,
        "bk": rng.standard_normal((M,), dtype=np.float32) * 0.04,
        "Wb": rng.standard_normal((1, H), dtype=np.float32) * 0.04,
        "bb": rng.standard_normal((1,), dtype=np.float32) * 0.04,
        "Wg": rng.standard_normal((1, H), dtype=np.float32) * 0.04,
        "bg": rng.standard_normal((1,), dtype=np.float32) * 0.04,
        "Ws": rng.standard_normal((3, H), dtype=np.float32) * 0.04,
        "bs": rng.standard_normal((3,), dtype=np.float32) * 0.04,
        "Wm": rng.standard_normal((1, H), dtype=np.float32) * 0.04,
        "bm": rng.standard_normal((1,), dtype=np.float32) * 0.04,
    }
    o = kernel(**ins)
    print("kernel output", o.shape, o.dtype, o.sum())


# revision 7
# speedup vs baseline: 2.5861x; 2.5861x over previous
"""Trainium2 Bass kernel for nn_Head (NTM-style addressing head).

Data-parallel over batch: 8 cores x 128 samples. Each core computes its
[128, 16384] slice of the output with zero collectives.

Self-contained: only imports numpy + the installed concourse stack.
"""

import sys

if "/opt/trn_rl_repo" not in sys.path:
    sys.path.insert(0, "/opt/trn_rl_repo")

from contextlib import ExitStack

import numpy as np

import concourse.bass as bass
import concourse.tile as tile
from concourse import bacc, mybir
from concourse.bass_utils import run_bass_kernel_spmd
from concourse.masks import make_identity

B, H, N, M = 1024, 512, 16384, 64
NCORES = 8
BL = B // NCORES  # 128 samples per core == partition count
HALF = N // 2  # 8192
NCH = 8  # elementwise chunks over N
CH = N // NCH  # 2048
F32 = mybir.dt.float32
BF16 = mybir.dt.bfloat16
AF = mybir.ActivationFunctionType
OP = mybir.AluOpType
AX = mybir.AxisListType
GMAX = 1.0 - 2.0**-23  # clamp for g so (1-g) stays representable


def _body(ctx: ExitStack, tc: tile.TileContext, out, h, wprev, m, wcat, bcat):
    nc = tc.nc

    const = ctx.enter_context(tc.tile_pool(name="const", bufs=1))
    small = ctx.enter_context(tc.tile_pool(name="small", bufs=1))
    mpool = ctx.enter_context(tc.tile_pool(name="mpool", bufs=1))
    big = ctx.enter_context(tc.tile_pool(name="big", bufs=1))
    psum = ctx.enter_context(tc.tile_pool(name="psum", bufs=2, space="PSUM"))

    # ---- constants ----
    ident = const.tile([128, 128], F32, name="ident")
    make_identity(nc, ident)
    bias_sb = const.tile([128, 70], F32, name="bias_sb")
    nc.gpsimd.dma_start(out=bias_sb, in_=bcat.to_broadcast((128, 70)))
    wcat_sb = const.tile([128, 4, 70], F32, name="wcat_sb")
    nc.sync.dma_start(out=wcat_sb, in_=wcat.rearrange("(j p) c -> p j c", p=128))

    # ---- h -> hT (4 PE transposes packed into one PSUM tile) ----
    h_sb = small.tile([128, H], F32, name="h_sb")
    nc.sync.dma_start(out=h_sb, in_=h)
    ps_h = psum.tile([128, CH], F32, name="ps_h", tag="ps")
    for j in range(4):
        nc.tensor.transpose(
            ps_h[:, 128 * j : 128 * (j + 1)], h_sb[:, 128 * j : 128 * (j + 1)], ident
        )
    hT = const.tile([128, 512], F32, name="hT")
    nc.scalar.copy(hT, ps_h[:, 0:512])

    # ---- fused head projections: proj = h @ Wcat + bcat  [128, 70] ----
    ps_p = psum.tile([128, CH], F32, name="ps_p", tag="ps")
    for j in range(4):
        nc.tensor.matmul(
            ps_p[:, 0:70],
            lhsT=hT[:, 128 * j : 128 * (j + 1)],
            rhs=wcat_sb[:, j, :],
            start=(j == 0),
            stop=(j == 3),
        )
    proj = small.tile([128, 70], F32, name="proj")
    nc.vector.tensor_tensor(out=proj, in0=ps_p[:, 0:70], in1=bias_sb, op=OP.add)

    # ---- head activations ----
    k_sb = small.tile([128, 64], F32, name="k_sb")
    nc.vector.tensor_scalar(
        out=k_sb, in0=proj[:, 0:64], scalar1=0.0, scalar2=1.0, op0=OP.max, op1=OP.min
    )
    beta = small.tile([128, 1], F32, name="beta")
    nc.vector.tensor_scalar(
        out=beta, in0=proj[:, 64:65], scalar1=0.0, scalar2=None, op0=OP.max
    )
    g_sb = small.tile([128, 1], F32, name="g_sb")
    nc.vector.tensor_scalar(
        out=g_sb, in0=proj[:, 65:66], scalar1=0.0, scalar2=1.0, op0=OP.max, op1=OP.min
    )
    gamma = small.tile([128, 1], F32, name="gamma")
    nc.vector.tensor_scalar(
        out=gamma, in0=proj[:, 69:70], scalar1=0.0, scalar2=1.0, op0=OP.max, op1=OP.add
    )
    # shift softmax over 3 logits (small range -> no max subtraction)
    es = small.tile([128, 3], F32, name="es")
    ssum = small.tile([128, 1], F32, name="ssum")
    nc.scalar.activation(out=es, in_=proj[:, 66:69], func=AF.Exp, accum_out=ssum)
    lss = small.tile([128, 1], F32, name="lss")
    nc.scalar.activation(out=lss, in_=ssum, func=AF.Ln)
    iss = small.tile([128, 1], F32, name="iss")
    nc.scalar.activation(out=iss, in_=lss, func=AF.Exp, scale=-1.0)
    s3 = small.tile([128, 3], F32, name="s3")
    nc.vector.tensor_scalar(out=s3, in0=es, scalar1=iss, scalar2=None, op0=OP.mult)

    # ---- bscale = beta / ||k||  (squares on DVE; rsqrt via exp(-0.5 ln)) ----
    scratch64 = small.tile([128, 64], F32, name="scratch64")
    nc.vector.tensor_tensor(out=scratch64, in0=k_sb, in1=k_sb, op=OP.mult)
    ksq = small.tile([128, 1], F32, name="ksq")
    nc.vector.tensor_reduce(out=ksq, in_=scratch64, axis=AX.X, op=OP.add)
    kln = small.tile([128, 1], F32, name="kln")
    nc.scalar.activation(out=kln, in_=ksq, func=AF.Ln)
    invk = small.tile([128, 1], F32, name="invk")
    nc.scalar.activation(out=invk, in_=kln, func=AF.Exp, scale=-0.5)
    bscale = small.tile([128, 1], F32, name="bscale")
    nc.vector.tensor_tensor(out=bscale, in0=beta, in1=invk, op=OP.mult)

    # ---- kT (bf16), duplicated on both partition halves: transpose [k | k] ----
    k2 = small.tile([128, 128], F32, name="k2")
    nc.vector.tensor_copy(k2[:, 0:64], k_sb)
    nc.vector.tensor_copy(k2[:, 64:128], k_sb)
    ps_k = psum.tile([128, CH], F32, name="ps_k", tag="ps")
    nc.tensor.transpose(ps_k[:, 0:128], k2, ident)
    kT = const.tile([128, 128], BF16, name="kT")
    nc.scalar.copy(kT, ps_k[:, 0:128])

    # ---- m_t: normalize rows, transpose into mT bf16 [128, 8192]
    #      rows 0:64  = m^T for n in [0, 8192)
    #      rows 64:128= m^T for n in [8192, 16384) ----
    mT = big.tile([128, HALF], BF16, name="mT")
    for half_batch in range(2):  # two batches of two pair-tiles
        pes = []
        es_rng = [2 * half_batch, 2 * half_batch + 1]
        for e in es_rng:
            pe = mpool.tile([128, 16, 2, 64], F32, name=f"mpair{e % 2}", tag=f"mp{e % 2}")
            lo = m[e * 2048 : (e + 1) * 2048, :].rearrange("(c p) m -> p c m", p=128)
            hi = m[HALF + e * 2048 : HALF + (e + 1) * 2048, :].rearrange(
                "(c p) m -> p c m", p=128
            )
            nc.sync.dma_start(out=pe[:, :, 0, :], in_=lo)
            nc.sync.dma_start(out=pe[:, :, 1, :], in_=hi)
            pes.append(pe)
        nrm2s, invns = [], []
        for i, e in enumerate(es_rng):
            ps_q = psum.tile([128, CH], F32, name="ps_q", tag="ps")
            nc.vector.tensor_tensor(out=ps_q, in0=pes[i], in1=pes[i], op=OP.mult)
            nrm2 = mpool.tile([128, 16, 2], F32, name=f"nrm2_{e % 2}", tag=f"n{e % 2}")
            nc.vector.tensor_reduce(
                out=nrm2,
                in_=ps_q.rearrange("p (c h m) -> p c h m", c=16, h=2),
                axis=AX.X,
                op=OP.add,
            )
            nrm2s.append(nrm2)
        for i, e in enumerate(es_rng):  # batched Ln
            lnn = mpool.tile([128, 16, 2], F32, name=f"lnn_{e % 2}", tag=f"l{e % 2}")
            nc.scalar.activation(out=lnn, in_=nrm2s[i], func=AF.Ln)
            invns.append(lnn)
        for i, e in enumerate(es_rng):  # batched Exp
            nc.scalar.activation(out=invns[i], in_=invns[i], func=AF.Exp, scale=-0.5)
        for i, e in enumerate(es_rng):
            nc.vector.tensor_tensor(
                out=pes[i],
                in0=pes[i],
                in1=invns[i].broadcast_to((128, 16, 2, 64)),
                op=OP.mult,
            )
            ps_m = psum.tile([128, CH], F32, name="ps_m", tag="ps")
            for c in range(16):
                nc.tensor.transpose(
                    ps_m[:, c * 128 : (c + 1) * 128], pes[i][:, c, :, :], ident
                )
            if e % 2 == 0:
                nc.vector.tensor_copy(mT[:, e * 2048 : (e + 1) * 2048], ps_m)
            else:
                nc.scalar.copy(mT[:, e * 2048 : (e + 1) * 2048], ps_m)

    # ---- w_tm1 in one big DMA ----
    wp_sb = big.tile([128, N], F32, name="wp_sb")
    nc.sync.dma_start(out=wp_sb, in_=wprev)

    # ---- content scores + exp (u = exp(beta * cos)), row sums via accum ----
    u_sb = big.tile([128, N], F32, name="u_sb")
    racc = small.tile([128, NCH], F32, name="racc")
    for j in range(NCH):
        ps_n = psum.tile([128, CH], F32, name="ps_n", tag="ps")
        if j < 4:
            kTv, rows, base = kT[0:64, :], slice(0, 64), j * CH
        else:
            kTv, rows, base = kT[64:128, :], slice(64, 128), (j - 4) * CH
        for b in range(4):
            nc.tensor.matmul(
                ps_n[:, b * 512 : (b + 1) * 512],
                lhsT=kTv,
                rhs=mT[rows, base + b * 512 : base + (b + 1) * 512],
                start=True,
                stop=True,
            )
        nc.scalar.activation(
            out=u_sb[:, j * CH : (j + 1) * CH],
            in_=ps_n,
            func=AF.Exp,
            scale=bscale,
            accum_out=racc[:, j : j + 1],
        )

    # ---- R = sum(u); gs = g / ((1-g_clamped) * R) ----
    R = small.tile([128, 1], F32, name="R")
    nc.vector.tensor_reduce(out=R, in_=racc, axis=AX.X, op=OP.add)
    gcl = small.tile([128, 1], F32, name="gcl")
    nc.vector.tensor_scalar(out=gcl, in0=g_sb, scalar1=GMAX, scalar2=None, op0=OP.min)
    om = small.tile([128, 1], F32, name="om")
    nc.vector.tensor_scalar(
        out=om, in0=gcl, scalar1=-1.0, scalar2=1.0, op0=OP.mult, op1=OP.add
    )
    omr = small.tile([128, 1], F32, name="omr")
    nc.vector.tensor_tensor(out=omr, in0=om, in1=R, op=OP.mult)
    lomr = small.tile([128, 1], F32, name="lomr")
    nc.scalar.activation(out=lomr, in_=omr, func=AF.Ln)
    iomr = small.tile([128, 1], F32, name="iomr")
    nc.scalar.activation(out=iomr, in_=lomr, func=AF.Exp, scale=-1.0)
    gs = small.tile([128, 1], F32, name="gs")
    nc.vector.tensor_tensor(out=gs, in0=g_sb, in1=iomr, op=OP.mult)

    # ---- v_hat = gs*u + w_tm1  -> bf16 ----
    vb = big.tile([128, N], BF16, name="vb")
    for j in range(NCH):
        sl = slice(j * CH, (j + 1) * CH)
        nc.vector.scalar_tensor_tensor(
            out=vb[:, sl],
            in0=u_sb[:, sl],
            scalar=gs,
            in1=wp_sb[:, sl],
            op0=OP.mult,
            op1=OP.add,
        )

    # ---- per-sample diag(s_j) stationaries (bf16) for the shift conv ----
    diag = const.tile([128, 3, 128], BF16, name="diag")
    for t in range(3):
        nc.vector.tensor_scalar(
            out=diag[:, t, :], in0=ident, scalar1=s3[:, t : t + 1], scalar2=None,
            op0=OP.mult,
        )

    # ---- conv (PE bf16, accumulating taps) -> ln (batched) ----
    eps_b = small.tile([128, 1], F32, name="eps_b")
    nc.vector.memset(eps_b, 1e-6)
    l_sb = u_sb  # reuse: u dead after v_hat
    for j in range(NCH):
        ps_c = psum.tile([128, CH], F32, name="ps_c", tag="ps")
        # tap order [1, 0, 2]: tap 1 (no shift) covers every column -> start
        for ti, t in enumerate([1, 0, 2]):
            for b in range(4):
                o0, o1 = b * 512, (b + 1) * 512
                r0 = j * CH + b * 512 + t - 1
                r1 = r0 + 512
                if r0 < 0:
                    o0, r0 = o0 + 1, 0
                if r1 > N:
                    o1, r1 = o1 - 1, N
                nc.tensor.matmul(
                    ps_c[:, o0:o1],
                    lhsT=diag[:, t, :],
                    rhs=vb[:, r0:r1],
                    start=(ti == 0),
                    stop=(ti == 2),
                    skip_group_check=True,
                )
        nc.scalar.activation(
            out=l_sb[:, j * CH : (j + 1) * CH],
            in_=ps_c,
            func=AF.Ln,
            scale=om,
            bias=eps_b,
        )

    # ---- sharpen: w = exp(gamma * l) (batched Exp), row sums via accum ----
    w_sb = wp_sb  # reuse: w_tm1 dead after v_hat
    sacc = small.tile([128, NCH], F32, name="sacc")
    for j in range(NCH):
        sl = slice(j * CH, (j + 1) * CH)
        nc.scalar.activation(
            out=w_sb[:, sl],
            in_=l_sb[:, sl],
            func=AF.Exp,
            scale=gamma,
            accum_out=sacc[:, j : j + 1],
        )

    # ---- final normalize + store ----
    Ssum = small.tile([128, 1], F32, name="Ssum")
    nc.vector.tensor_reduce(out=Ssum, in_=sacc, axis=AX.X, op=OP.add)
    lS = small.tile([128, 1], F32, name="lS")
    nc.scalar.activation(out=lS, in_=Ssum, func=AF.Ln)
    sinv = small.tile([128, 1], F32, name="sinv")
    nc.scalar.activation(out=sinv, in_=lS, func=AF.Exp, scale=-1.0)
    for j in range(NCH):
        sl = slice(j * CH, (j + 1) * CH)
        nc.vector.tensor_scalar(
            out=w_sb[:, sl], in0=w_sb[:, sl], scalar1=sinv, scalar2=None, op0=OP.mult
        )
        nc.sync.dma_start(out=out[:, sl], in_=w_sb[:, sl])


def build_program():
    nc = bacc.Bacc(
        "TRN2", target_bir_lowering=False, debug=False, num_devices=NCORES
    )
    h = nc.dram_tensor("h", [BL, H], F32, kind="ExternalInput").ap()
    wprev = nc.dram_tensor("wprev", [BL, N], F32, kind="ExternalInput").ap()
    m = nc.dram_tensor("m", [N, M], F32, kind="ExternalInput").ap()
    wcat = nc.dram_tensor("wcat", [H, 70], F32, kind="ExternalInput").ap()
    bcat = nc.dram_tensor("bcat", [1, 70], F32, kind="ExternalInput").ap()
    out = nc.dram_tensor("out", [BL, N], F32, kind="ExternalOutput").ap()
    with tile.TileContext(nc) as tc, ExitStack() as ctx:
        _body(ctx, tc, out, h, wprev, m, wcat, bcat)
    nc.compile()
    return nc


_CACHED_NC = None


def _pack_host_inputs(
    h_t, w_tm1, m_t, Wk, bk, Wb, bb, Wg, bg, Ws, bs, Wm, bm
):
    wcat = np.concatenate([Wk, Wb, Wg, Ws, Wm], axis=0).astype(np.float32)  # [70, H]
    bcat = np.concatenate([bk, bb, bg, bs, bm], axis=0).astype(np.float32)  # [70]
    wcat_t = np.ascontiguousarray(wcat.T)  # [H, 70]
    bcat2 = np.ascontiguousarray(bcat.reshape(1, 70))
    m_c = np.ascontiguousarray(np.asarray(m_t, dtype=np.float32))
    in_maps = []
    for c in range(NCORES):
        sl = slice(c * BL, (c + 1) * BL)
        in_maps.append(
            {
                "h": np.ascontiguousarray(np.asarray(h_t[sl], dtype=np.float32)),
                "wprev": np.ascontiguousarray(np.asarray(w_tm1[sl], dtype=np.float32)),
                "m": m_c,
                "wcat": wcat_t,
                "bcat": bcat2,
            }
        )
    return in_maps


def kernel(**inputs) -> np.ndarray:
    global _CACHED_NC
    if _CACHED_NC is None:
        _CACHED_NC = build_program()
    in_maps = _pack_host_inputs(**inputs)
    res = run_bass_kernel_spmd(_CACHED_NC, in_maps, core_ids=list(range(NCORES)))
    return np.concatenate([res.results[c]["out"] for c in range(NCORES)], axis=0)


# revision 10
# speedup vs baseline: 2.9235x; 1.1305x over previous
"""Trainium2 Bass kernel for nn_Head (NTM-style addressing head).

Data-parallel over batch: 8 cores x 128 samples. Each core computes its
[128, 16384] slice of the output with zero collectives.

Self-contained: only imports numpy + the installed concourse stack.
"""

import sys

if "/opt/trn_rl_repo" not in sys.path:
    sys.path.insert(0, "/opt/trn_rl_repo")

from contextlib import ExitStack

import numpy as np

import concourse.bass as bass
import concourse.tile as tile
from concourse import bacc, mybir
from concourse.bass_utils import run_bass_kernel_spmd
from concourse.masks import make_identity
from concourse.tile import add_dep_helper

B, H, N, M = 1024, 512, 16384, 64
NCORES = 8
BL = B // NCORES  # 128 samples per core == partition count
HALF = N // 2  # 8192
NCH = 8  # elementwise chunks over N
CH = N // NCH  # 2048
F32 = mybir.dt.float32
BF16 = mybir.dt.bfloat16
AF = mybir.ActivationFunctionType
OP = mybir.AluOpType
AX = mybir.AxisListType
GMAX = 1.0 - 2.0**-23  # clamp for g so (1-g) stays representable


def _body(ctx: ExitStack, tc: tile.TileContext, out, h, wprev, mtp, wcat, bcat):
    nc = tc.nc

    const = ctx.enter_context(tc.tile_pool(name="const", bufs=1))
    small = ctx.enter_context(tc.tile_pool(name="small", bufs=1))
    big = ctx.enter_context(tc.tile_pool(name="big", bufs=1))
    sq_pool = ctx.enter_context(tc.tile_pool(name="sq_pool", bufs=2))
    psum = ctx.enter_context(tc.tile_pool(name="psum", bufs=2, space="PSUM"))

    # ---- constants ----
    ident = const.tile([128, 128], F32, name="ident")
    make_identity(nc, ident)
    bias_sb = const.tile([128, 70], F32, name="bias_sb")
    nc.gpsimd.dma_start(out=bias_sb, in_=bcat.to_broadcast((128, 70)))
    wcat_sb = const.tile([128, 4, 70], F32, name="wcat_sb")
    nc.sync.dma_start(out=wcat_sb, in_=wcat.rearrange("(j p) c -> p j c", p=128))
    ones2 = const.tile([128, 64], BF16, name="ones2")
    nc.vector.memset(ones2, 1.0)

    # ---- h -> hT (4 PE transposes packed into one PSUM tile) ----
    h_sb = small.tile([128, H], F32, name="h_sb")
    nc.sync.dma_start(out=h_sb, in_=h)
    ps_h = psum.tile([128, CH], F32, name="ps_h", tag="ps")
    for j in range(4):
        nc.tensor.transpose(
            ps_h[:, 128 * j : 128 * (j + 1)], h_sb[:, 128 * j : 128 * (j + 1)], ident
        )
    hT = const.tile([128, 512], F32, name="hT")
    nc.vector.tensor_copy(hT, ps_h[:, 0:512])

    # ---- fused head projections: proj = h @ Wcat + bcat  [128, 70] ----
    ps_p = psum.tile([128, CH], F32, name="ps_p", tag="ps")
    for j in range(4):
        nc.tensor.matmul(
            ps_p[:, 0:70],
            lhsT=hT[:, 128 * j : 128 * (j + 1)],
            rhs=wcat_sb[:, j, :],
            start=(j == 0),
            stop=(j == 3),
        )
    proj = small.tile([128, 70], F32, name="proj")
    nc.vector.tensor_tensor(out=proj, in0=ps_p[:, 0:70], in1=bias_sb, op=OP.add)

    # ---- head activations ----
    k_sb = small.tile([128, 64], F32, name="k_sb")
    nc.vector.tensor_scalar(
        out=k_sb, in0=proj[:, 0:64], scalar1=0.0, scalar2=1.0, op0=OP.max, op1=OP.min
    )
    beta = small.tile([128, 1], F32, name="beta")
    nc.vector.tensor_scalar(
        out=beta, in0=proj[:, 64:65], scalar1=0.0, scalar2=None, op0=OP.max
    )
    g_sb = small.tile([128, 1], F32, name="g_sb")
    nc.vector.tensor_scalar(
        out=g_sb, in0=proj[:, 65:66], scalar1=0.0, scalar2=1.0, op0=OP.max, op1=OP.min
    )
    gamma = small.tile([128, 1], F32, name="gamma")
    nc.vector.tensor_scalar(
        out=gamma, in0=proj[:, 69:70], scalar1=0.0, scalar2=1.0, op0=OP.max, op1=OP.add
    )
    # shift softmax over 3 logits (small range -> no max subtraction)
    es = small.tile([128, 3], F32, name="es")
    ssum = small.tile([128, 1], F32, name="ssum")
    nc.scalar.activation(out=es, in_=proj[:, 66:69], func=AF.Exp, accum_out=ssum)
    iss = small.tile([128, 1], F32, name="iss")
    nc.vector.reciprocal(out=iss, in_=ssum)
    s3 = small.tile([128, 3], F32, name="s3")
    nc.vector.tensor_scalar(out=s3, in0=es, scalar1=iss, scalar2=None, op0=OP.mult)

    # ---- bscale = beta / ||k||  (squares on DVE; rsqrt via exp(-0.5 ln)) ----
    scratch64 = small.tile([128, 64], F32, name="scratch64")
    nc.vector.tensor_tensor(out=scratch64, in0=k_sb, in1=k_sb, op=OP.mult)
    ksq = small.tile([128, 1], F32, name="ksq")
    nc.vector.tensor_reduce(out=ksq, in_=scratch64, axis=AX.X, op=OP.add)
    kln = small.tile([128, 1], F32, name="kln")
    nc.scalar.activation(out=kln, in_=ksq, func=AF.Ln)
    invk = small.tile([128, 1], F32, name="invk")
    nc.scalar.activation(out=invk, in_=kln, func=AF.Exp, scale=-0.5)
    bscale = small.tile([128, 1], F32, name="bscale")
    nc.vector.tensor_tensor(out=bscale, in0=beta, in1=invk, op=OP.mult)

    # ---- kT (bf16), duplicated on both partition halves: transpose [k | k] ----
    k2 = small.tile([128, 128], F32, name="k2")
    nc.vector.tensor_copy(k2[:, 0:64], k_sb)
    nc.vector.tensor_copy(k2[:, 64:128], k_sb)
    ps_k = psum.tile([128, CH], F32, name="ps_k", tag="ps")
    nc.tensor.transpose(ps_k[:, 0:128], k2, ident)
    kT = const.tile([128, 128], BF16, name="kT")
    nc.vector.tensor_copy(kT, ps_k[:, 0:128])

    # ---- m: host provides mtp [128, 8192] f32:
    #      rows 0:64  = m^T for n in [0, 8192)
    #      rows 64:128= m^T for n in [8192, 16384)
    #   normalize columns by per-n 1/||m_n|| -> mT_s bf16 ----
    mp = big.tile([128, HALF], F32, name="mp", tag="mp_vb")
    nc.sync.dma_start(out=mp, in_=mtp)
    mT = big.tile([128, HALF], BF16, name="mT")
    m_exps = []
    for grp in range(2):  # 2 chunks per group: exactly the 2 PSUM slots
        cs = [2 * grp, 2 * grp + 1]
        inv_pss = []
        for c in cs:
            sl = slice(c * 2048, (c + 1) * 2048)
            psq = sq_pool.tile([128, 2048], BF16, name="psq", tag="psq")
            nc.vector.tensor_tensor(out=psq, in0=mp[:, sl], in1=mp[:, sl], op=OP.mult)
            inv_ps = psum.tile([128, CH], F32, name="inv_ps", tag="ps")
            for s in range(4):
                ssl = slice(s * 512, (s + 1) * 512)
                nc.tensor.matmul(
                    inv_ps[0:64, ssl], lhsT=ones2[0:64, :], rhs=psq[0:64, ssl],
                    start=True, stop=True,
                )
                nc.tensor.matmul(
                    inv_ps[64:128, ssl], lhsT=ones2[64:128, :], rhs=psq[64:128, ssl],
                    start=True, stop=True, tile_position=(64, 64),
                )
            inv_pss.append(inv_ps)
        m_lns = []
        for i, c in enumerate(cs):  # batched Ln (in-place on PSUM)
            i1 = nc.scalar.activation(out=inv_pss[i], in_=inv_pss[i], func=AF.Ln)
            m_lns.append(i1)
        for i, c in enumerate(cs):  # batched Exp -> invn = nrm2^-0.5 (in-place)
            i2 = nc.scalar.activation(
                out=inv_pss[i], in_=inv_pss[i], func=AF.Exp, scale=-0.5
            )
            if i == 0:
                add_dep_helper(i2.ins, m_lns[-1].ins, sync=False, reason="act batch")
            m_exps.append(i2)
        for i, c in enumerate(cs):
            sl = slice(c * 2048, (c + 1) * 2048)
            nc.vector.tensor_tensor(
                out=mT[:, sl], in0=mp[:, sl], in1=inv_pss[i], op=OP.mult
            )

    # ---- w_tm1 in one big DMA ----
    wp_sb = big.tile([128, N], F32, name="wp_sb")
    nc.sync.dma_start(out=wp_sb, in_=wprev)

    # ---- content scores + exp (u = exp(beta * cos)), row sums via accum ----
    u_sb = big.tile([128, N], F32, name="u_sb")
    racc = small.tile([128, NCH], F32, name="racc")
    u_exps = []
    for j in range(NCH):
        ps_n = psum.tile([128, CH], F32, name="ps_n", tag="ps")
        if j < 4:
            kTv, rows, base = kT[0:64, :], slice(0, 64), j * CH
        else:
            kTv, rows, base = kT[64:128, :], slice(64, 128), (j - 4) * CH
        for b in range(4):
            nc.tensor.matmul(
                ps_n[:, b * 512 : (b + 1) * 512],
                lhsT=kTv,
                rhs=mT[rows, base + b * 512 : base + (b + 1) * 512],
                start=True,
                stop=True,
            )
        ue = nc.scalar.activation(
            out=u_sb[:, j * CH : (j + 1) * CH],
            in_=ps_n,
            func=AF.Exp,
            scale=bscale,
            accum_out=racc[:, j : j + 1],
        )
        u_exps.append(ue)
    add_dep_helper(u_exps[0].ins, m_exps[-1].ins, sync=False, reason="act batch")

    # ---- R = sum(u); gs = g / ((1-g_clamped) * R) ----
    R = small.tile([128, 1], F32, name="R")
    nc.vector.tensor_reduce(out=R, in_=racc, axis=AX.X, op=OP.add)
    gcl = small.tile([128, 1], F32, name="gcl")
    nc.vector.tensor_scalar(out=gcl, in0=g_sb, scalar1=GMAX, scalar2=None, op0=OP.min)
    om = small.tile([128, 1], F32, name="om")
    nc.vector.tensor_scalar(
        out=om, in0=gcl, scalar1=-1.0, scalar2=1.0, op0=OP.mult, op1=OP.add
    )
    omr = small.tile([128, 1], F32, name="omr")
    nc.vector.tensor_tensor(out=omr, in0=om, in1=R, op=OP.mult)
    iomr = small.tile([128, 1], F32, name="iomr")
    nc.vector.reciprocal(out=iomr, in_=omr)
    gs = small.tile([128, 1], F32, name="gs")
    nc.vector.tensor_tensor(out=gs, in0=g_sb, in1=iomr, op=OP.mult)

    # ---- v_hat = gs*u + w_tm1  -> bf16 (slot shared with mp) ----
    vb = big.tile([128, N], BF16, name="vb", tag="mp_vb")
    for j in range(NCH):
        sl = slice(j * CH, (j + 1) * CH)
        nc.vector.scalar_tensor_tensor(
            out=vb[:, sl],
            in0=u_sb[:, sl],
            scalar=gs,
            in1=wp_sb[:, sl],
            op0=OP.mult,
            op1=OP.add,
        )

    # ---- per-sample diag(s_j) stationaries (bf16) for the shift conv ----
    diag = const.tile([128, 3, 128], BF16, name="diag")
    for t in range(3):
        nc.vector.tensor_scalar(
            out=diag[:, t, :], in0=ident, scalar1=s3[:, t : t + 1], scalar2=None,
            op0=OP.mult,
        )

    # ---- conv (PE bf16, accumulating taps) -> ln (batched) ----
    eps_b = small.tile([128, 1], F32, name="eps_b")
    nc.vector.memset(eps_b, 1e-6)
    l_sb = u_sb  # reuse: u dead after v_hat
    lns = []
    for j in range(NCH):
        ps_c = psum.tile([128, CH], F32, name="ps_c", tag="ps")
        # tap order [1, 0, 2]: tap 1 (no shift) covers every column -> start
        for ti, t in enumerate([1, 0, 2]):
            for b in range(4):
                o0, o1 = b * 512, (b + 1) * 512
                r0 = j * CH + b * 512 + t - 1
                r1 = r0 + 512
                if r0 < 0:
                    o0, r0 = o0 + 1, 0
                if r1 > N:
                    o1, r1 = o1 - 1, N
                nc.tensor.matmul(
                    ps_c[:, o0:o1],
                    lhsT=diag[:, t, :],
                    rhs=vb[:, r0:r1],
                    start=(ti == 0),
                    stop=(ti == 2),
                    skip_group_check=True,
                )
        li = nc.scalar.activation(
            out=l_sb[:, j * CH : (j + 1) * CH],
            in_=ps_c,
            func=AF.Ln,
            scale=om,
            bias=eps_b,
        )
        lns.append(li)
    add_dep_helper(lns[0].ins, u_exps[-1].ins, sync=False, reason="act batch")

    # ---- sharpen: w = exp(gamma * l) (batched Exp), row sums via accum ----
    w_sb = wp_sb  # reuse: w_tm1 dead after v_hat
    sacc = small.tile([128, NCH], F32, name="sacc")
    exp2s = []
    for j in range(NCH):
        sl = slice(j * CH, (j + 1) * CH)
        e2 = nc.scalar.activation(
            out=w_sb[:, sl],
            in_=l_sb[:, sl],
            func=AF.Exp,
            scale=gamma,
            accum_out=sacc[:, j : j + 1],
        )
        exp2s.append(e2)
    add_dep_helper(exp2s[0].ins, lns[-1].ins, sync=False, reason="act batch")

    # ---- final normalize (split DVE/ACT) + store ----
    Ssum = small.tile([128, 1], F32, name="Ssum")
    nc.vector.tensor_reduce(out=Ssum, in_=sacc, axis=AX.X, op=OP.add)
    sinv = small.tile([128, 1], F32, name="sinv")
    nc.vector.reciprocal(out=sinv, in_=Ssum)
    for j in range(NCH):
        sl = slice(j * CH, (j + 1) * CH)
        if j % 2 == 0:
            nc.vector.tensor_scalar(
                out=w_sb[:, sl], in0=w_sb[:, sl], scalar1=sinv, scalar2=None,
                op0=OP.mult,
            )
        else:
            nc.scalar.activation(
                out=w_sb[:, sl], in_=w_sb[:, sl], func=AF.Copy, scale=sinv
            )
        nc.sync.dma_start(out=out[:, sl], in_=w_sb[:, sl])


def build_program():
    nc = bacc.Bacc(
        "TRN2", target_bir_lowering=False, debug=False, num_devices=NCORES
    )
    h = nc.dram_tensor("h", [BL, H], F32, kind="ExternalInput").ap()
    wprev = nc.dram_tensor("wprev", [BL, N], F32, kind="ExternalInput").ap()
    mtp = nc.dram_tensor("mtp", [128, HALF], F32, kind="ExternalInput").ap()
    wcat = nc.dram_tensor("wcat", [H, 70], F32, kind="ExternalInput").ap()
    bcat = nc.dram_tensor("bcat", [1, 70], F32, kind="ExternalInput").ap()
    out = nc.dram_tensor("out", [BL, N], F32, kind="ExternalOutput").ap()
    with tile.TileContext(nc) as tc, ExitStack() as ctx:
        _body(ctx, tc, out, h, wprev, mtp, wcat, bcat)
    nc.compile()
    return nc


_CACHED_NC = None


def _pack_host_inputs(
    h_t, w_tm1, m_t, Wk, bk, Wb, bb, Wg, bg, Ws, bs, Wm, bm
):
    wcat = np.concatenate([Wk, Wb, Wg, Ws, Wm], axis=0).astype(np.float32)  # [70, H]
    bcat = np.concatenate([bk, bb, bg, bs, bm], axis=0).astype(np.float32)  # [70]
    wcat_t = np.ascontiguousarray(wcat.T)  # [H, 70]
    bcat2 = np.ascontiguousarray(bcat.reshape(1, 70))
    mt = np.asarray(m_t, dtype=np.float32).T  # [64, N] view
    mtp = np.ascontiguousarray(
        np.concatenate([mt[:, :HALF], mt[:, HALF:]], axis=0)
    )  # [128, 8192]
    in_maps = []
    for c in range(NCORES):
        sl = slice(c * BL, (c + 1) * BL)
        in_maps.append(
            {
                "h": np.ascontiguousarray(np.asarray(h_t[sl], dtype=np.float32)),
                "wprev": np.ascontiguousarray(np.asarray(w_tm1[sl], dtype=np.float32)),
                "mtp": mtp,
                "wcat": wcat_t,
                "bcat": bcat2,
            }
        )
    return in_maps


def kernel(**inputs) -> np.ndarray:
    global _CACHED_NC
    if _CACHED_NC is None:
        _CACHED_NC = build_program()
    in_maps = _pack_host_inputs(**inputs)
    res = run_bass_kernel_spmd(_CACHED_NC, in_maps, core_ids=list(range(NCORES)))
    return np.concatenate([res.results[c]["out"] for c in range(NCORES)], axis=0)


# revision 13
# speedup vs baseline: 3.1191x; 1.0669x over previous
"""Trainium2 Bass kernel for nn_Head (NTM-style addressing head).

Data-parallel over batch: 8 cores x 128 samples. Each core computes its
[128, 16384] slice of the output with zero collectives.

Self-contained: only imports numpy + the installed concourse stack.
"""

import sys

if "/opt/trn_rl_repo" not in sys.path:
    sys.path.insert(0, "/opt/trn_rl_repo")

from contextlib import ExitStack

import numpy as np

import concourse.bass as bass
import concourse.tile as tile
from concourse import bacc, mybir
from concourse.bass_utils import run_bass_kernel_spmd
from concourse.masks import make_identity
from concourse.tile import add_dep_helper

B, H, N, M = 1024, 512, 16384, 64
NCORES = 8
BL = B // NCORES  # 128 samples per core == partition count
HALF = N // 2  # 8192
NCH = 8  # elementwise chunks over N
CH = N // NCH  # 2048
F32 = mybir.dt.float32
BF16 = mybir.dt.bfloat16
AF = mybir.ActivationFunctionType
OP = mybir.AluOpType
AX = mybir.AxisListType
GMAX = 1.0 - 2.0**-23  # clamp for g so (1-g) stays representable


def _body(ctx: ExitStack, tc: tile.TileContext, out, h, wprev, mtp, wcat, bcat):
    nc = tc.nc

    const = ctx.enter_context(tc.tile_pool(name="const", bufs=1))
    small = ctx.enter_context(tc.tile_pool(name="small", bufs=1))
    big = ctx.enter_context(tc.tile_pool(name="big", bufs=1))
    sq_pool = ctx.enter_context(tc.tile_pool(name="sq_pool", bufs=2))
    psum = ctx.enter_context(tc.tile_pool(name="psum", bufs=2, space="PSUM"))

    # ---- constants ----
    ident = const.tile([128, 128], F32, name="ident")
    make_identity(nc, ident)
    bias_sb = const.tile([128, 70], F32, name="bias_sb")
    nc.gpsimd.dma_start(out=bias_sb, in_=bcat.to_broadcast((128, 70)))
    wcat_sb = const.tile([128, 4, 70], F32, name="wcat_sb")
    nc.sync.dma_start(out=wcat_sb, in_=wcat.rearrange("(j p) c -> p j c", p=128))
    ones2 = const.tile([128, 64], BF16, name="ones2")
    nc.vector.memset(ones2, 1.0)

    # ---- h -> hT (4 PE transposes packed into one PSUM tile) ----
    h_sb = small.tile([128, H], F32, name="h_sb")
    nc.sync.dma_start(out=h_sb, in_=h)
    ps_h = psum.tile([128, CH], F32, name="ps_h", tag="ps")
    for j in range(4):
        nc.tensor.transpose(
            ps_h[:, 128 * j : 128 * (j + 1)], h_sb[:, 128 * j : 128 * (j + 1)], ident
        )
    hT = const.tile([128, 512], F32, name="hT")
    nc.vector.tensor_copy(hT, ps_h[:, 0:512])

    # ---- fused head projections: proj = h @ Wcat + bcat  [128, 70] ----
    ps_p = psum.tile([128, CH], F32, name="ps_p", tag="ps")
    for j in range(4):
        nc.tensor.matmul(
            ps_p[:, 0:70],
            lhsT=hT[:, 128 * j : 128 * (j + 1)],
            rhs=wcat_sb[:, j, :],
            start=(j == 0),
            stop=(j == 3),
        )
    proj = small.tile([128, 70], F32, name="proj")
    nc.vector.tensor_tensor(out=proj, in0=ps_p[:, 0:70], in1=bias_sb, op=OP.add)

    # ---- head activations ----
    k_sb = small.tile([128, 64], F32, name="k_sb")
    nc.vector.tensor_scalar(
        out=k_sb, in0=proj[:, 0:64], scalar1=0.0, scalar2=1.0, op0=OP.max, op1=OP.min
    )
    beta = small.tile([128, 1], F32, name="beta")
    nc.vector.tensor_scalar(
        out=beta, in0=proj[:, 64:65], scalar1=0.0, scalar2=None, op0=OP.max
    )
    g_sb = small.tile([128, 1], F32, name="g_sb")
    nc.vector.tensor_scalar(
        out=g_sb, in0=proj[:, 65:66], scalar1=0.0, scalar2=1.0, op0=OP.max, op1=OP.min
    )
    gamma = small.tile([128, 1], F32, name="gamma")
    nc.vector.tensor_scalar(
        out=gamma, in0=proj[:, 69:70], scalar1=0.0, scalar2=1.0, op0=OP.max, op1=OP.add
    )
    # shift softmax over 3 logits (small range -> no max subtraction)
    es = small.tile([128, 3], F32, name="es")
    ssum = small.tile([128, 1], F32, name="ssum")
    nc.scalar.activation(out=es, in_=proj[:, 66:69], func=AF.Exp, accum_out=ssum)
    iss = small.tile([128, 1], F32, name="iss")
    nc.vector.reciprocal(out=iss, in_=ssum)
    s3 = small.tile([128, 3], F32, name="s3")
    nc.vector.tensor_scalar(out=s3, in0=es, scalar1=iss, scalar2=None, op0=OP.mult)

    # ---- bscale = beta / ||k||  (squares on DVE; rsqrt via exp(-0.5 ln)) ----
    scratch64 = small.tile([128, 64], F32, name="scratch64")
    nc.vector.tensor_tensor(out=scratch64, in0=k_sb, in1=k_sb, op=OP.mult)
    ksq = small.tile([128, 1], F32, name="ksq")
    nc.vector.tensor_reduce(out=ksq, in_=scratch64, axis=AX.X, op=OP.add)
    kln = small.tile([128, 1], F32, name="kln")
    nc.scalar.activation(out=kln, in_=ksq, func=AF.Ln)
    invk = small.tile([128, 1], F32, name="invk")
    nc.scalar.activation(out=invk, in_=kln, func=AF.Exp, scale=-0.5)
    bscale = small.tile([128, 1], F32, name="bscale")
    nc.vector.tensor_tensor(out=bscale, in0=beta, in1=invk, op=OP.mult)

    # ---- kT (bf16), duplicated on both partition halves: transpose [k | k] ----
    k2 = small.tile([128, 128], F32, name="k2")
    nc.vector.tensor_copy(k2[:, 0:64], k_sb)
    nc.vector.tensor_copy(k2[:, 64:128], k_sb)
    ps_k = psum.tile([128, CH], F32, name="ps_k", tag="ps")
    nc.tensor.transpose(ps_k[:, 0:128], k2, ident)
    kT = const.tile([128, 128], BF16, name="kT")
    nc.vector.tensor_copy(kT, ps_k[:, 0:128])

    # ---- m: host provides mtp [128, 8192] f32:
    #      rows 0:64  = m^T for n in [0, 8192)
    #      rows 64:128= m^T for n in [8192, 16384)
    #   normalize columns by per-n 1/||m_n|| -> mT_s bf16 ----
    mp = big.tile([128, HALF], F32, name="mp", tag="mp_vb")
    nc.sync.dma_start(out=mp, in_=mtp)
    mT = big.tile([128, HALF], BF16, name="mT")
    m_exps = []
    for grp in range(2):  # 2 chunks per group: exactly the 2 PSUM slots
        cs = [2 * grp, 2 * grp + 1]
        inv_pss = []
        for c in cs:
            sl = slice(c * 2048, (c + 1) * 2048)
            psq = sq_pool.tile([128, 2048], BF16, name="psq", tag="psq")
            nc.vector.tensor_tensor(out=psq, in0=mp[:, sl], in1=mp[:, sl], op=OP.mult)
            inv_ps = psum.tile([128, CH], F32, name="inv_ps", tag="ps")
            for s in range(4):
                ssl = slice(s * 512, (s + 1) * 512)
                nc.tensor.matmul(
                    inv_ps[0:64, ssl], lhsT=ones2[0:64, :], rhs=psq[0:64, ssl],
                    start=True, stop=True,
                )
                nc.tensor.matmul(
                    inv_ps[64:128, ssl], lhsT=ones2[64:128, :], rhs=psq[64:128, ssl],
                    start=True, stop=True, tile_position=(64, 64),
                )
            inv_pss.append(inv_ps)
        m_lns = []
        for i, c in enumerate(cs):  # batched Ln (in-place on PSUM)
            i1 = nc.scalar.activation(out=inv_pss[i], in_=inv_pss[i], func=AF.Ln)
            m_lns.append(i1)
        for i, c in enumerate(cs):  # batched Exp -> invn = nrm2^-0.5 (in-place)
            i2 = nc.scalar.activation(
                out=inv_pss[i], in_=inv_pss[i], func=AF.Exp, scale=-0.5
            )
            add_dep_helper(i2.ins, m_lns[-1].ins, sync=False, reason="act batch")
            m_exps.append(i2)
        for i, c in enumerate(cs):
            sl = slice(c * 2048, (c + 1) * 2048)
            nc.vector.tensor_tensor(
                out=mT[:, sl], in0=mp[:, sl], in1=inv_pss[i], op=OP.mult
            )

    # ---- w_tm1 in one big DMA ----
    wp_sb = big.tile([128, N], F32, name="wp_sb")
    nc.sync.dma_start(out=wp_sb, in_=wprev)

    # ---- content scores + exp (u = exp(beta * cos)), row sums via accum ----
    u_sb = big.tile([128, N], F32, name="u_sb")
    racc = small.tile([128, NCH], F32, name="racc")
    u_exps = []
    for j in range(NCH):
        ps_n = psum.tile([128, CH], F32, name="ps_n", tag="ps")
        if j < 4:
            kTv, rows, base = kT[0:64, :], slice(0, 64), j * CH
        else:
            kTv, rows, base = kT[64:128, :], slice(64, 128), (j - 4) * CH
        for b in range(4):
            nc.tensor.matmul(
                ps_n[:, b * 512 : (b + 1) * 512],
                lhsT=kTv,
                rhs=mT[rows, base + b * 512 : base + (b + 1) * 512],
                start=True,
                stop=True,
            )
        ue = nc.scalar.activation(
            out=u_sb[:, j * CH : (j + 1) * CH],
            in_=ps_n,
            func=AF.Exp,
            scale=bscale,
            accum_out=racc[:, j : j + 1],
        )
        u_exps.append(ue)
    add_dep_helper(u_exps[0].ins, m_exps[-1].ins, sync=False, reason="act batch")

    # ---- R = sum(u); gs = g / ((1-g_clamped) * R) ----
    R = small.tile([128, 1], F32, name="R")
    nc.vector.tensor_reduce(out=R, in_=racc, axis=AX.X, op=OP.add)
    gcl = small.tile([128, 1], F32, name="gcl")
    nc.vector.tensor_scalar(out=gcl, in0=g_sb, scalar1=GMAX, scalar2=None, op0=OP.min)
    om = small.tile([128, 1], F32, name="om")
    nc.vector.tensor_scalar(
        out=om, in0=gcl, scalar1=-1.0, scalar2=1.0, op0=OP.mult, op1=OP.add
    )
    omr = small.tile([128, 1], F32, name="omr")
    nc.vector.tensor_tensor(out=omr, in0=om, in1=R, op=OP.mult)
    iomr = small.tile([128, 1], F32, name="iomr")
    nc.vector.reciprocal(out=iomr, in_=omr)
    gs = small.tile([128, 1], F32, name="gs")
    nc.vector.tensor_tensor(out=gs, in0=g_sb, in1=iomr, op=OP.mult)

    # ---- v_hat = gs*u + w_tm1  -> bf16 (slot shared with mp) ----
    vb = big.tile([128, N], BF16, name="vb", tag="mp_vb")
    stts = []
    for j in range(NCH):
        sl = slice(j * CH, (j + 1) * CH)
        stts.append(nc.vector.scalar_tensor_tensor(
            out=vb[:, sl],
            in0=u_sb[:, sl],
            scalar=gs,
            in1=wp_sb[:, sl],
            op0=OP.mult,
            op1=OP.add,
        ))

    # keep the PE HAM warm across the R-barrier: dummy weight loads tied
    # to the v_hat chain so the scheduler spreads them through the gap
    for j in range(NCH):
        dw = nc.tensor.ldweights(kT[0:64, :])
        add_dep_helper(dw.ins, stts[j].ins, sync=False, reason="ham warm")

    # ---- per-sample diag(s_j) stationaries (bf16) for the shift conv ----
    diag = const.tile([128, 3, 128], BF16, name="diag")
    for t in range(3):
        nc.vector.tensor_scalar(
            out=diag[:, t, :], in0=ident, scalar1=s3[:, t : t + 1], scalar2=None,
            op0=OP.mult,
        )

    # ---- conv (PE bf16, accumulating taps) -> ln (batched) ----
    eps_b = small.tile([128, 1], F32, name="eps_b")
    nc.vector.memset(eps_b, 1e-6)
    l_sb = u_sb  # reuse: u dead after v_hat
    lns = []
    for j in range(NCH):
        ps_c = psum.tile([128, CH], F32, name="ps_c", tag="ps")
        # tap order [1, 0, 2]: tap 1 (no shift) covers every column -> start
        for ti, t in enumerate([1, 0, 2]):
            for b in range(4):
                o0, o1 = b * 512, (b + 1) * 512
                r0 = j * CH + b * 512 + t - 1
                r1 = r0 + 512
                if r0 < 0:
                    o0, r0 = o0 + 1, 0
                if r1 > N:
                    o1, r1 = o1 - 1, N
                nc.tensor.matmul(
                    ps_c[:, o0:o1],
                    lhsT=diag[:, t, :],
                    rhs=vb[:, r0:r1],
                    start=(ti == 0),
                    stop=(ti == 2),
                    skip_group_check=True,
                )
        li = nc.scalar.activation(
            out=l_sb[:, j * CH : (j + 1) * CH],
            in_=ps_c,
            func=AF.Ln,
            scale=om,
            bias=eps_b,
        )
        add_dep_helper(li.ins, u_exps[-1].ins, sync=False, reason="act batch")
        lns.append(li)

    # ---- sharpen: w = exp(gamma * l) (batched Exp), row sums via accum ----
    w_sb = wp_sb  # reuse: w_tm1 dead after v_hat
    sacc = small.tile([128, NCH], F32, name="sacc")
    exp2s = []
    for j in range(NCH):
        sl = slice(j * CH, (j + 1) * CH)
        e2 = nc.scalar.activation(
            out=w_sb[:, sl],
            in_=l_sb[:, sl],
            func=AF.Exp,
            scale=gamma,
            accum_out=sacc[:, j : j + 1],
        )
        add_dep_helper(e2.ins, lns[-1].ins, sync=False, reason="act batch")
        exp2s.append(e2)

    # ---- final normalize (split DVE/ACT) + store ----
    Ssum = small.tile([128, 1], F32, name="Ssum")
    nc.vector.tensor_reduce(out=Ssum, in_=sacc, axis=AX.X, op=OP.add)
    sinv = small.tile([128, 1], F32, name="sinv")
    nc.vector.reciprocal(out=sinv, in_=Ssum)
    for j in range(NCH):
        sl = slice(j * CH, (j + 1) * CH)
        nc.vector.tensor_scalar(
            out=w_sb[:, sl], in0=w_sb[:, sl], scalar1=sinv, scalar2=None,
            op0=OP.mult,
        )
        nc.sync.dma_start(out=out[:, sl], in_=w_sb[:, sl])


def build_program():
    nc = bacc.Bacc(
        "TRN2", target_bir_lowering=False, debug=False, num_devices=NCORES
    )
    h = nc.dram_tensor("h", [BL, H], F32, kind="ExternalInput").ap()
    wprev = nc.dram_tensor("wprev", [BL, N], F32, kind="ExternalInput").ap()
    mtp = nc.dram_tensor("mtp", [128, HALF], F32, kind="ExternalInput").ap()
    wcat = nc.dram_tensor("wcat", [H, 70], F32, kind="ExternalInput").ap()
    bcat = nc.dram_tensor("bcat", [1, 70], F32, kind="ExternalInput").ap()
    out = nc.dram_tensor("out", [BL, N], F32, kind="ExternalOutput").ap()
    with tile.TileContext(nc) as tc, ExitStack() as ctx:
        _body(ctx, tc, out, h, wprev, mtp, wcat, bcat)
    nc.compile()
    return nc


_CACHED_NC = None


def _pack_host_inputs(
    h_t, w_tm1, m_t, Wk, bk, Wb, bb, Wg, bg, Ws, bs, Wm, bm
):
    wcat = np.concatenate([Wk, Wb, Wg, Ws, Wm], axis=0).astype(np.float32)  # [70, H]
    bcat = np.concatenate([bk, bb, bg, bs, bm], axis=0).astype(np.float32)  # [70]
    wcat_t = np.ascontiguousarray(wcat.T)  # [H, 70]
    bcat2 = np.ascontiguousarray(bcat.reshape(1, 70))
    mt = np.asarray(m_t, dtype=np.float32).T  # [64, N] view
    mtp = np.ascontiguousarray(
        np.concatenate([mt[:, :HALF], mt[:, HALF:]], axis=0)
    )  # [128, 8192]
    in_maps = []
    for c in range(NCORES):
        sl = slice(c * BL, (c + 1) * BL)
        in_maps.append(
            {
                "h": np.ascontiguousarray(np.asarray(h_t[sl], dtype=np.float32)),
                "wprev": np.ascontiguousarray(np.asarray(w_tm1[sl], dtype=np.float32)),
                "mtp": mtp,
                "wcat": wcat_t,
                "bcat": bcat2,
            }
        )
    return in_maps


def kernel(**inputs) -> np.ndarray:
    global _CACHED_NC
    if _CACHED_NC is None:
        _CACHED_NC = build_program()
    in_maps = _pack_host_inputs(**inputs)
    res = run_bass_kernel_spmd(_CACHED_NC, in_maps, core_ids=list(range(NCORES)))
    return np.concatenate([res.results[c]["out"] for c in range(NCORES)], axis=0)
